# revision 1
# baseline (speedup 1.0000x reference)
"""Multi-Head Latent Attention on 8 Trainium2 NeuronCores.

Sharding: core c = (batch b = c//4) x (head-group g = c%4, 4 heads each).
Each core computes the down-projections for its batch (replicated within
the 4-core batch group), up-projections/rope/attention for its 4 heads,
and a partial output projection. Host sums the 4 partials per batch and
adds the output bias (plus the value-up bias folded through out_w, which
is exact because softmax rows sum to 1).

All on-device layouts are feature-major ("transposed"): x^T, kvq_c^T,
K^T, Q^T, ctx^T, out^T. This makes every matmul contraction land on the
partition axis with zero transposes. Scores are computed as
scores^T[k, q] so that probs^T feeds the context matmul directly; the
softmax denominator comes from a ones-vector matmul (partition-axis sum
on the PE), and exp is applied without max-subtraction (scores for this
problem are in [-1, 1], verified offline).

Rope is applied via the "swapped-weight" identity:
  rot(Wx + b) = cos .* (Wx + b) + sin .* (W_swap x + b_swap)
with W_swap column pairs (w_{2i}, w_{2i+1}) -> (-w_{2i+1}, w_{2i}), which
keeps everything partition-aligned (no cross-partition reads).
"""

import numpy as np
import ml_dtypes

import concourse.bass as bass
import concourse.mybir as mybir
from concourse.tile import TileContext
from concourse.bass_utils import run_bass_kernel_spmd

F32 = mybir.dt.float32
BF16 = mybir.dt.bfloat16
AF = mybir.ActivationFunctionType
BF = ml_dtypes.bfloat16

HIDDEN = 2048
NUM_HEADS = 16
HEAD_DIM = 128
KV_C = 512
Q_C = 1536
ROPE_DIM = 64
B, S = 2, 2048

P = 128
NH = 4          # heads per core
SC = 512        # free-dim chunk for projections / q-chunks
NKT = HIDDEN // P       # 16 k-tiles of the down projection
NMD = HIDDEN // P       # 16 output chunks of the down projection (kv+q)
SCALE = float(1.0 / np.sqrt(HEAD_DIM + ROPE_DIM))
NEG = -1.0e5


def _split_waits(nc, maxw=1):
    """This container's walrus accepts at most one sem-wait per instruction;
    move excess waits onto same-engine NOPs inserted immediately before."""
    for fn in nc.m.functions:
        for bb in fn.blocks:
            newlist = []
            for ins in bb.instructions:
                si = ins.sync_info
                if si is not None and si.on_wait is not None and len(si.on_wait) > maxw:
                    waits = list(si.on_wait)
                    extra, keep = waits[:-maxw], waits[-maxw:]
                    for k, i in enumerate(range(0, len(extra), maxw)):
                        nop = mybir.InstNoOp(
                            name=f"{ins.name}-waitsplit-{k}", ins=[], outs=[]
                        )
                        nop.engine = ins.engine
                        nop.sync_info = mybir.SyncInfo(
                            on_wait=extra[i : i + maxw], on_update=[]
                        )
                        newlist.append(nop)
                    ins.sync_info = mybir.SyncInfo(
                        on_wait=keep, on_update=list(si.on_update or [])
                    )
                newlist.append(ins)
            bb.instructions = newlist


def build(debug=False):
    nc = bass.Bass()
    dt = nc.dram_tensor
    xT = dt("xT", [HIDDEN, S], BF16, kind="ExternalInput")
    Wd = dt("Wd", [HIDDEN, KV_C + Q_C], BF16, kind="ExternalInput")
    bd = dt("bd", [P, NMD], F32, kind="ExternalInput")
    Wku = dt("Wku", [KV_C, NH * HEAD_DIM], BF16, kind="ExternalInput")
    bku = dt("bku", [P, 4], F32, kind="ExternalInput")
    Wvu = dt("Wvu", [KV_C, NH * HEAD_DIM], BF16, kind="ExternalInput")
    Wkr = dt("Wkr", [KV_C, NH * ROPE_DIM], BF16, kind="ExternalInput")
    Wkrs = dt("Wkrs", [KV_C, NH * ROPE_DIM], BF16, kind="ExternalInput")
    bkr = dt("bkr", [P, 2], F32, kind="ExternalInput")
    bkrs = dt("bkrs", [P, 2], F32, kind="ExternalInput")
    Wqu = dt("Wqu", [Q_C, NH * HEAD_DIM], BF16, kind="ExternalInput")
    bqu = dt("bqu", [P, 4], F32, kind="ExternalInput")
    Wqr = dt("Wqr", [Q_C, NH * ROPE_DIM], BF16, kind="ExternalInput")
    Wqrs = dt("Wqrs", [Q_C, NH * ROPE_DIM], BF16, kind="ExternalInput")
    bqr = dt("bqr", [P, 2], F32, kind="ExternalInput")
    bqrs = dt("bqrs", [P, 2], F32, kind="ExternalInput")
    Wo = dt("Wo", [NH * HEAD_DIM, HIDDEN], BF16, kind="ExternalInput")
    cos2 = dt("cos2", [P, S], BF16, kind="ExternalInput")
    sin2 = dt("sin2", [P, S], BF16, kind="ExternalInput")
    tri = dt("tri", [P, P], F32, kind="ExternalInput")
    outT = dt("outT", [HIDDEN, S], F32, kind="ExternalOutput")
    if debug:
        dbg_kvq = dt("dbg_kvq", [HIDDEN, S], BF16, kind="ExternalOutput")
        dbg_kc = dt("dbg_kc", [NH * HEAD_DIM, S], BF16, kind="ExternalOutput")
        dbg_kr = dt("dbg_kr", [2 * P, S], BF16, kind="ExternalOutput")
        dbg_qc = dt("dbg_qc", [NH * HEAD_DIM, S], BF16, kind="ExternalOutput")
        dbg_qr = dt("dbg_qr", [2 * P, S], BF16, kind="ExternalOutput")
        dbg_v = dt("dbg_v", [S, NH * HEAD_DIM], BF16, kind="ExternalOutput")

    NSC = S // SC  # 4 free-dim chunks

    with TileContext(nc) as tc:
        with (
            tc.tile_pool(name="const", bufs=1) as pc,
            tc.tile_pool(name="kvq", bufs=1) as pkvq,
        ):
            # --- constants (gpsimd DMA queue; keep sync queue free for xT) ---
            cos_sb = pc.tile([P, S], BF16)
            sin_sb = pc.tile([P, S], BF16)
            tri_sb = pc.tile([P, P], F32)
            nc.gpsimd.dma_start(tri_sb[:], tri[:])
            bd_sb = pc.tile([P, NMD], F32)
            nc.gpsimd.dma_start(bd_sb[:], bd[:])
            bku_sb = pc.tile([P, 4], F32)
            nc.gpsimd.dma_start(bku_sb[:], bku[:])
            bkr_sb = pc.tile([P, 2], F32)
            nc.gpsimd.dma_start(bkr_sb[:], bkr[:])
            bkrs_sb = pc.tile([P, 2], F32)
            nc.gpsimd.dma_start(bkrs_sb[:], bkrs[:])
            bqu_sb = pc.tile([P, 4], F32)
            nc.gpsimd.dma_start(bqu_sb[:], bqu[:])
            bqr_sb = pc.tile([P, 2], F32)
            nc.gpsimd.dma_start(bqr_sb[:], bqr[:])
            bqrs_sb = pc.tile([P, 2], F32)
            nc.gpsimd.dma_start(bqrs_sb[:], bqrs[:])
            ones_mat = pc.tile([P, P], BF16)
            nc.vector.memset(ones_mat[:], 1.0)
            ones_row = pc.tile([1, P], BF16)
            nc.vector.memset(ones_row[:], 1.0)

            kvq_sb = pkvq.tile([P, NKT, S], BF16)

            # ---------------- phase 1: down projection ----------------
            with (
                tc.tile_pool(name="p1", bufs=1) as p1,
                tc.tile_pool(name="p1w", bufs=3) as p1w,
                tc.tile_pool(name="ps1", bufs=4, space="PSUM") as ps1,
            ):
                xTr = xT.rearrange("(t p) s -> p t s", p=P)
                xt_tiles = []
                for k in range(NKT):
                    t = p1.tile([P, S], BF16, tag=f"xt{k}")
                    nc.sync.dma_start(t[:], xTr[:, k, :])
                    xt_tiles.append(t)
                for m in range(NMD):
                    wd_t = p1w.tile([P, NKT, P], BF16, tag="wd")
                    nc.gpsimd.dma_start(
                        wd_t[:],
                        Wd[:, m * P : (m + 1) * P].rearrange(
                            "(t p) m -> p t m", p=P
                        ),
                    )
                    for s in range(NSC):
                        ps = ps1.tile([P, SC], F32, tag="mm")
                        for k in range(NKT):
                            nc.tensor.matmul(
                                ps[:],
                                wd_t[:, k, :],
                                xt_tiles[k][:, s * SC : (s + 1) * SC],
                                start=(k == 0),
                                stop=(k == NKT - 1),
                            )
                        nc.vector.tensor_scalar_add(
                            kvq_sb[:, m, s * SC : (s + 1) * SC],
                            ps[:],
                            bd_sb[:, m : m + 1],
                        )
                nc.gpsimd.dma_start(cos_sb[:], cos2[:])
                nc.gpsimd.dma_start(sin_sb[:], sin2[:])

            if debug:
                nc.sync.dma_start(
                    dbg_kvq.rearrange("(t p) s -> p t s", p=P), kvq_sb[:]
                )

            # ------------- phase 2: up projections + rope -------------
            with tc.tile_pool(name="qkv", bufs=1) as pq:
                kc_sb = pq.tile([P, NH, S], BF16)
                kr_sb = pq.tile([P, 2, S], BF16)
                qc_sb = pq.tile([P, NH, S], BF16)
                qr_sb = pq.tile([P, 2, S], BF16)
                v_sb = pq.tile([P, S // P, NH * HEAD_DIM], BF16)

                with (
                    tc.tile_pool(name="p2w", bufs=2) as p2w,
                    tc.tile_pool(name="p2t", bufs=3) as p2t,
                    tc.tile_pool(name="ps2", bufs=4, space="PSUM") as ps2,
                ):
                    # K_c^T: 4 chunks of 128 head-features
                    for m in range(NH):
                        wt = p2w.tile([P, 4, P], BF16, tag="wku")
                        nc.sync.dma_start(
                            wt[:],
                            Wku[:, m * P : (m + 1) * P].rearrange(
                                "(t p) m -> p t m", p=P
                            ),
                        )
                        for s in range(NSC):
                            ps = ps2.tile([P, SC], F32, tag="mm")
                            for k in range(4):
                                nc.tensor.matmul(
                                    ps[:],
                                    wt[:, k, :],
                                    kvq_sb[:, k, s * SC : (s + 1) * SC],
                                    start=(k == 0),
                                    stop=(k == 3),
                                )
                            nc.vector.tensor_scalar_add(
                                kc_sb[:, m, s * SC : (s + 1) * SC],
                                ps[:],
                                bku_sb[:, m : m + 1],
                            )
                    # V token-major
                    wv_t = p2w.tile([P, 4, NH * HEAD_DIM], BF16, tag="wvu")
                    nc.sync.dma_start(
                        wv_t[:], Wvu.rearrange("(t p) m -> p t m", p=P)
                    )
                    for t in range(S // P):
                        ps = ps2.tile([P, NH * HEAD_DIM], F32, tag="mm")
                        for k in range(4):
                            nc.tensor.matmul(
                                ps[:],
                                kvq_sb[:, k, t * P : (t + 1) * P],
                                wv_t[:, k, :],
                                start=(k == 0),
                                stop=(k == 3),
                            )
                        nc.vector.tensor_copy(v_sb[:, t, :], ps[:])

                    # K rope (swapped-weight trick), chunks of 2 heads
                    for m in range(2):
                        wa = p2w.tile([P, 4, P], BF16, tag="wkr")
                        nc.sync.dma_start(
                            wa[:],
                            Wkr[:, m * P : (m + 1) * P].rearrange(
                                "(t p) m -> p t m", p=P
                            ),
                        )
                        wb = p2w.tile([P, 4, P], BF16, tag="wkrs")
                        nc.sync.dma_start(
                            wb[:],
                            Wkrs[:, m * P : (m + 1) * P].rearrange(
                                "(t p) m -> p t m", p=P
                            ),
                        )
                        for s in range(NSC):
                            sl = slice(s * SC, (s + 1) * SC)
                            psA = ps2.tile([P, SC], F32, tag="mm")
                            for k in range(4):
                                nc.tensor.matmul(
                                    psA[:], wa[:, k, :], kvq_sb[:, k, sl],
                                    start=(k == 0), stop=(k == 3),
                                )
                            psB = ps2.tile([P, SC], F32, tag="mm")
                            for k in range(4):
                                nc.tensor.matmul(
                                    psB[:], wb[:, k, :], kvq_sb[:, k, sl],
                                    start=(k == 0), stop=(k == 3),
                                )
                            tA = p2t.tile([P, SC], F32, tag="ropeA")
                            nc.vector.tensor_scalar_add(
                                tA[:], psA[:], bkr_sb[:, m : m + 1]
                            )
                            tB = p2t.tile([P, SC], F32, tag="ropeB")
                            nc.vector.tensor_scalar_add(
                                tB[:], psB[:], bkrs_sb[:, m : m + 1]
                            )
                            nc.vector.tensor_tensor(
                                tA[:], tA[:], cos_sb[:, sl],
                                mybir.AluOpType.mult,
                            )
                            nc.vector.tensor_tensor(
                                tB[:], tB[:], sin_sb[:, sl],
                                mybir.AluOpType.mult,
                            )
                            nc.vector.tensor_tensor(
                                kr_sb[:, m, sl], tA[:], tB[:],
                                mybir.AluOpType.add,
                            )

                    # Q_c^T
                    for m in range(NH):
                        wt = p2w.tile([P, 12, P], BF16, tag="wqu")
                        nc.sync.dma_start(
                            wt[:],
                            Wqu[:, m * P : (m + 1) * P].rearrange(
                                "(t p) m -> p t m", p=P
                            ),
                        )
                        for s in range(NSC):
                            ps = ps2.tile([P, SC], F32, tag="mm")
                            for k in range(12):
                                nc.tensor.matmul(
                                    ps[:],
                                    wt[:, k, :],
                                    kvq_sb[:, 4 + k, s * SC : (s + 1) * SC],
                                    start=(k == 0),
                                    stop=(k == 11),
                                )
                            nc.vector.tensor_scalar_add(
                                qc_sb[:, m, s * SC : (s + 1) * SC],
                                ps[:],
                                bqu_sb[:, m : m + 1],
                            )
                    # Q rope
                    for m in range(2):
                        wa = p2w.tile([P, 12, P], BF16, tag="wqr")
                        nc.sync.dma_start(
                            wa[:],
                            Wqr[:, m * P : (m + 1) * P].rearrange(
                                "(t p) m -> p t m", p=P
                            ),
                        )
                        wb = p2w.tile([P, 12, P], BF16, tag="wqrs")
                        nc.sync.dma_start(
                            wb[:],
                            Wqrs[:, m * P : (m + 1) * P].rearrange(
                                "(t p) m -> p t m", p=P
                            ),
                        )
                        for s in range(NSC):
                            sl = slice(s * SC, (s + 1) * SC)
                            psA = ps2.tile([P, SC], F32, tag="mm")
                            for k in range(12):
                                nc.tensor.matmul(
                                    psA[:], wa[:, k, :], kvq_sb[:, 4 + k, sl],
                                    start=(k == 0), stop=(k == 11),
                                )
                            psB = ps2.tile([P, SC], F32, tag="mm")
                            for k in range(12):
                                nc.tensor.matmul(
                                    psB[:], wb[:, k, :], kvq_sb[:, 4 + k, sl],
                                    start=(k == 0), stop=(k == 11),
                                )
                            tA = p2t.tile([P, SC], F32, tag="ropeA")
                            nc.vector.tensor_scalar_add(
                                tA[:], psA[:], bqr_sb[:, m : m + 1]
                            )
                            tB = p2t.tile([P, SC], F32, tag="ropeB")
                            nc.vector.tensor_scalar_add(
                                tB[:], psB[:], bqrs_sb[:, m : m + 1]
                            )
                            nc.vector.tensor_tensor(
                                tA[:], tA[:], cos_sb[:, sl],
                                mybir.AluOpType.mult,
                            )
                            nc.vector.tensor_tensor(
                                tB[:], tB[:], sin_sb[:, sl],
                                mybir.AluOpType.mult,
                            )
                            nc.vector.tensor_tensor(
                                qr_sb[:, m, sl], tA[:], tB[:],
                                mybir.AluOpType.add,
                            )

                if debug:
                    nc.sync.dma_start(
                        dbg_kc.rearrange("(t p) s -> p t s", p=P), kc_sb[:]
                    )
                    nc.sync.dma_start(
                        dbg_kr.rearrange("(t p) s -> p t s", p=P), kr_sb[:]
                    )
                    nc.sync.dma_start(
                        dbg_qc.rearrange("(t p) s -> p t s", p=P), qc_sb[:]
                    )
                    nc.sync.dma_start(
                        dbg_qr.rearrange("(t p) s -> p t s", p=P), qr_sb[:]
                    )
                    nc.sync.dma_start(
                        dbg_v.rearrange("(t p) d -> p t d", p=P), v_sb[:]
                    )

                # ---------- phase 3: attention + inline out-proj ----------
                with (
                    tc.tile_pool(name="at", bufs=8) as pat,
                    tc.tile_pool(name="atx", bufs=2) as patx,
                    tc.tile_pool(name="att", bufs=2) as patt,
                    tc.tile_pool(name="out", bufs=3) as pout,
                    tc.tile_pool(name="ow", bufs=3) as pow_,
                    tc.tile_pool(name="ps_sc", bufs=2, space="PSUM") as ps_sc,
                    tc.tile_pool(name="ps_acc", bufs=2, space="PSUM") as ps_acc,
                    tc.tile_pool(name="ps_m", bufs=2, space="PSUM") as ps_m,
                ):
                    for qc in range(NSC):
                        qsl = slice(qc * SC, (qc + 1) * SC)
                        nkb = 4 * qc + 4
                        ctx_q = patx.tile([P, NH, SC], BF16, tag="ctx")
                        for h in range(NH):
                            hc = h // 2
                            hp = (h % 2) * ROPE_DIM
                            psum_ctx = ps_acc.tile([P, SC], F32, tag="ctx")
                            psum_sum = ps_acc.tile([P, SC], F32, tag="sum")
                            for kb in range(nkb):
                                ksl = slice(kb * P, (kb + 1) * P)
                                diag = kb >= 4 * qc
                                c = (kb - 4 * qc) * P if diag else 0
                                qs0 = qc * SC + c
                                ps = ps_sc.tile([P, SC], F32, tag="sc")
                                nc.tensor.matmul(
                                    ps[:, c:],
                                    kc_sb[:, h, ksl],
                                    qc_sb[:, h, qs0 : (qc + 1) * SC],
                                    start=True, stop=False,
                                )
                                nc.tensor.matmul(
                                    ps[:, c:],
                                    kr_sb[hp : hp + ROPE_DIM, hc, ksl],
                                    qr_sb[hp : hp + ROPE_DIM, hc,
                                          qs0 : (qc + 1) * SC],
                                    start=False, stop=True,
                                )
                                probs = pat.tile([P, SC], BF16, tag="probs")
                                if diag:
                                    nc.vector.tensor_tensor(
                                        ps[:, c : c + P],
                                        ps[:, c : c + P],
                                        tri_sb[:],
                                        mybir.AluOpType.add,
                                    )
                                nc.scalar.activation(
                                    probs[:, c:], ps[:, c:], AF.Exp,
                                    scale=SCALE,
                                )
                                nc.tensor.matmul(
                                    psum_sum[:, c:], ones_mat[:],
                                    probs[:, c:],
                                    start=(kb == 0), stop=(kb == nkb - 1),
                                )
                                nc.tensor.matmul(
                                    psum_ctx[:, c:],
                                    v_sb[:, kb, h * P : (h + 1) * P],
                                    probs[:, c:],
                                    start=(kb == 0), stop=(kb == nkb - 1),
                                )
                            sums_f = patt.tile([1, SC], F32, tag="sums")
                            nc.scalar.copy(sums_f[:], psum_sum[0:1, :])
                            r = patt.tile([1, SC], F32, tag="recip")
                            nc.vector.reciprocal(r[:], sums_f[:])
                            r16 = patt.tile([1, SC], BF16, tag="r16")
                            nc.vector.tensor_copy(r16[:], r[:])
                            psb = ps_m.tile([P, SC], F32, tag="m")
                            nc.tensor.matmul(
                                psb[:], ones_row[:], r16[:],
                                start=True, stop=True,
                            )
                            rbc = patt.tile([P, SC], BF16, tag="rbc")
                            nc.scalar.copy(rbc[:], psb[:])
                            nc.vector.tensor_tensor(
                                ctx_q[:, h, :], psum_ctx[:], rbc[:],
                                mybir.AluOpType.mult,
                            )

                        # out-projection for this q-chunk
                        for m in range(NMD):
                            wo_t = pow_.tile([P, NH, P], BF16, tag="wo")
                            nc.sync.dma_start(
                                wo_t[:],
                                Wo[:, m * P : (m + 1) * P].rearrange(
                                    "(t p) m -> p t m", p=P
                                ),
                            )
                            ps = ps_m.tile([P, SC], F32, tag="m")
                            for k in range(NH):
                                nc.tensor.matmul(
                                    ps[:],
                                    wo_t[:, k, :],
                                    ctx_q[:, k, :],
                                    start=(k == 0),
                                    stop=(k == NH - 1),
                                )
                            og = pout.tile([P, SC], F32, tag="og")
                            nc.scalar.copy(og[:], ps[:])
                            nc.sync.dma_start(
                                outT[m * P : (m + 1) * P,
                                     qc * SC : (qc + 1) * SC],
                                og[:],
                            )
    _split_waits(nc)
    return nc


def _swap_pairs(w):
    """(..., 2i) <- -(..., 2i+1); (..., 2i+1) <- (..., 2i) along last axis."""
    out = np.empty_like(w)
    out[..., 0::2] = -w[..., 1::2]
    out[..., 1::2] = w[..., 0::2]
    return out


def _col_bias(b, nm):
    """[nm*128] -> [128, nm] (column m = bias for feature chunk m)."""
    return np.ascontiguousarray(b.reshape(nm, P).T).astype(np.float32)


_NC = None


def kernel(**inputs):
    global _NC
    inp = {k: np.asarray(v) for k, v in inputs.items()}
    x = inp["x"].astype(np.float32)

    Wd_full = np.concatenate(
        [inp["kv_down_w"], inp["query_down_w"]], axis=1
    ).astype(BF)
    bd_full = np.concatenate([inp["kv_down_b"], inp["query_down_b"]])

    pos = np.arange(S, dtype=np.float64)
    inv = 1.0 / (10000.0 ** (np.arange(0, ROPE_DIM, 2, np.float64) / ROPE_DIM))
    ang = pos[None, :] * inv[:, None]          # [32, S]
    idx = (np.arange(P) % ROPE_DIM) // 2       # row -> freq index
    cos2 = np.cos(ang)[idx].astype(BF)
    sin2 = np.sin(ang)[idx].astype(BF)
    tri = np.where(
        np.arange(P)[None, :] >= np.arange(P)[:, None], 0.0, NEG
    ).astype(np.float32)

    in_maps = []
    for c in range(8):
        b, g = c // 4, c % 4
        h0 = g * NH
        csl = slice(h0 * HEAD_DIM, (h0 + NH) * HEAD_DIM)
        rsl = slice(h0 * ROPE_DIM, (h0 + NH) * ROPE_DIM)
        wkr = inp["key_rope_w"][:, rsl].astype(np.float32)
        wqr = inp["query_rope_w"][:, rsl].astype(np.float32)
        bkr = inp["key_rope_b"][rsl].astype(np.float32)
        bqr = inp["query_rope_b"][rsl].astype(np.float32)
        in_maps.append(
            {
                "xT": np.ascontiguousarray(x[b].T).astype(BF),
                "Wd": Wd_full,
                "bd": _col_bias(bd_full, NMD),
                "Wku": inp["key_up_w"][:, csl].astype(BF),
                "bku": _col_bias(inp["key_up_b"][csl], 4),
                "Wvu": inp["value_up_w"][:, csl].astype(BF),
                "Wkr": wkr.astype(BF),
                "Wkrs": _swap_pairs(wkr).astype(BF),
                "bkr": _col_bias(bkr, 2),
                "bkrs": _col_bias(_swap_pairs(bkr), 2),
                "Wqu": inp["query_up_w"][:, csl].astype(BF),
                "bqu": _col_bias(inp["query_up_b"][csl], 4),
                "Wqr": wqr.astype(BF),
                "Wqrs": _swap_pairs(wqr).astype(BF),
                "bqr": _col_bias(bqr, 2),
                "bqrs": _col_bias(_swap_pairs(bqr), 2),
                "Wo": inp["out_w"][csl, :].astype(BF),
                "cos2": cos2,
                "sin2": sin2,
                "tri": tri,
            }
        )

    if _NC is None:
        _NC = build()
    res = run_bass_kernel_spmd(_NC, in_maps, core_ids=list(range(8)))

    corr = (
        inp["value_up_b"].astype(np.float32) @ inp["out_w"].astype(np.float32)
        + inp["out_b"].astype(np.float32)
    )
    out = np.empty((B, S, HIDDEN), np.float32)
    for b in range(B):
        acc = res.results[b * 4]["outT"].copy()
        for g in range(1, 4):
            acc += res.results[b * 4 + g]["outT"]
        out[b] = acc.T + corr[None, :]
    return out



# revision 23
# speedup vs baseline: 1.0140x; 1.0140x over previous
"""Multi-Head Latent Attention on 8 Trainium2 NeuronCores.

Sharding: core c = (batch b = c//4) x (head-group g = c%4, 4 heads each).
The down projection is token-sharded across the 4 cores of a batch group:
core (b, r) computes kvq_c^T for token slices {512j + 128r : j=0..3} and
the full kvq is assembled with four pipelined AllGathers (one per
512-token chunk), overlapped with the down projection of later chunks and
with the per-chunk up-projections/attention that consume earlier chunks.
Each core computes up-projections/rope/attention for its 4 heads and a
partial output projection; the host sums the 4 bf16 partials per batch
and adds the output bias (plus the value-up bias folded through out_w,
which is exact because softmax rows sum to 1).

All on-device layouts are feature-major ("transposed"): x^T, kvq_c^T,
K^T, Q^T, ctx^T, out^T.  Scores are computed as scores^T[k, q] so that
probs^T feeds the context matmul directly.  Softmax denominators are
accumulated on the vector engine (probs summed across key blocks) and
reduced across partitions with a gpsimd partition_all_reduce, keeping the
tensor engine free.  Exp is applied without max-subtraction (scores for
this problem are in [-1, 1]).

Rope is applied token-major: the rope projections are computed once with
tokens on partitions (weights streaming), rotated pairwise on the vector
engine (pairs sit on the free axis), and transposed back to feature-major
with PE transposes.
"""

import numpy as np
import ml_dtypes

import concourse.bass as bass
import concourse.bass_isa as bass_isa
import concourse.mybir as mybir
from concourse.tile import TileContext
from concourse.bass_utils import run_bass_kernel_spmd

F32 = mybir.dt.float32
BF16 = mybir.dt.bfloat16
AF = mybir.ActivationFunctionType
BF = ml_dtypes.bfloat16

HIDDEN = 2048
NUM_HEADS = 16
HEAD_DIM = 128
KV_C = 512
Q_C = 1536
ROPE_DIM = 64
B, S = 2, 2048

P = 128
NH = 4          # heads per core
SC = 512        # token chunk (one AllGather / one attention q-chunk)
NKT = HIDDEN // P       # 16 k-tiles of the down projection
NMD = HIDDEN // P       # 16 output chunks of the down projection (kv+q)
NSC = S // SC           # 4 chunks
NT = S // P             # 16 token tiles
SCALE = float(1.0 / np.sqrt(HEAD_DIM + ROPE_DIM))
NEG = -1.0e5
GROUPS = [[0, 1, 2, 3], [4, 5, 6, 7]]
USE_PAR = False  # gpsimd partition_all_reduce unsupported by this walrus


def _split_waits(nc, maxw=1):
    """This container's walrus accepts at most one sem-wait per instruction;
    move excess waits onto same-engine NOPs inserted immediately before."""
    for fn in nc.m.functions:
        for bb in fn.blocks:
            newlist = []
            for ins in bb.instructions:
                si = ins.sync_info
                if si is not None and si.on_wait is not None and len(si.on_wait) > maxw:
                    waits = list(si.on_wait)
                    extra, keep = waits[:-maxw], waits[-maxw:]
                    for k, i in enumerate(range(0, len(extra), maxw)):
                        nop = mybir.InstNoOp(
                            name=f"{ins.name}-waitsplit-{k}", ins=[], outs=[]
                        )
                        nop.engine = ins.engine
                        nop.sync_info = mybir.SyncInfo(
                            on_wait=extra[i : i + maxw], on_update=[]
                        )
                        newlist.append(nop)
                    ins.sync_info = mybir.SyncInfo(
                        on_wait=keep, on_update=list(si.on_update or [])
                    )
                newlist.append(ins)
            bb.instructions = newlist


def build():
    nc = bass.Bass(num_devices=8)
    dt = nc.dram_tensor
    xTo = dt("xTo", [HIDDEN, NSC * P], BF16, kind="ExternalInput")
    Wd = dt("Wd", [HIDDEN, KV_C + Q_C], BF16, kind="ExternalInput")
    bd = dt("bd", [P, NMD], F32, kind="ExternalInput")
    Wku = dt("Wku", [KV_C, NH * HEAD_DIM], BF16, kind="ExternalInput")
    bku = dt("bku", [P, 4], F32, kind="ExternalInput")
    Wvu = dt("Wvu", [KV_C, NH * HEAD_DIM], BF16, kind="ExternalInput")
    Wkr = dt("Wkr", [KV_C, NH * ROPE_DIM], BF16, kind="ExternalInput")
    Wqu = dt("Wqu", [Q_C, NH * HEAD_DIM], BF16, kind="ExternalInput")
    bqu = dt("bqu", [P, 4], F32, kind="ExternalInput")
    Wqr = dt("Wqr", [Q_C, NH * ROPE_DIM], BF16, kind="ExternalInput")
    # rope biases, token-major broadcast: [P, NH*ROPE_DIM] (same per token)
    bkrT = dt("bkrT", [P, NH * ROPE_DIM], F32, kind="ExternalInput")
    bqrT = dt("bqrT", [P, NH * ROPE_DIM], F32, kind="ExternalInput")
    Wo = dt("Wo", [NH * HEAD_DIM, HIDDEN], BF16, kind="ExternalInput")
    # token-major cos/sin: [P tokens-within-tile, tile, NH, ROPE_DIM/2]
    cosT = dt("cosT", [P, NT * NH * (ROPE_DIM // 2)], BF16, kind="ExternalInput")
    sinT = dt("sinT", [P, NT * NH * (ROPE_DIM // 2)], BF16, kind="ExternalInput")
    tri = dt("tri", [P, P], F32, kind="ExternalInput")
    eye = dt("eye", [P, P], BF16, kind="ExternalInput")
    outT = dt("outT", [HIDDEN, S], BF16, kind="ExternalOutput")

    cc_in = [dt(f"cc_in{j}", [P, NKT, P], BF16, kind="Internal")
             for j in range(NSC)]
    cc_out = [dt(f"cc_out{j}", [4, P, NKT, P], BF16, kind="Internal")
              for j in range(NSC)]

    with TileContext(nc) as tc:
        with (
            tc.tile_pool(name="const", bufs=1) as pc,
            tc.tile_pool(name="kvq", bufs=1) as pkvq,
        ):
            # --- constants (gpsimd DMA queue; sync queue handles xTo) ---
            tri_sb = pc.tile([P, P], F32)
            nc.gpsimd.dma_start(tri_sb[:], tri[:])
            eye_sb = pc.tile([P, P], BF16)
            nc.gpsimd.dma_start(eye_sb[:], eye[:])
            ones_sb = pc.tile([P, P], BF16)
            nc.vector.memset(ones_sb[:], 1.0)
            bd_sb = pc.tile([P, NMD], F32)
            nc.gpsimd.dma_start(bd_sb[:], bd[:])
            bku_sb = pc.tile([P, 4], F32)
            nc.gpsimd.dma_start(bku_sb[:], bku[:])
            bqu_sb = pc.tile([P, 4], F32)
            nc.gpsimd.dma_start(bqu_sb[:], bqu[:])
            bkrT_sb = pc.tile([P, NH * ROPE_DIM], F32)
            nc.gpsimd.dma_start(bkrT_sb[:], bkrT[:])
            bqrT_sb = pc.tile([P, NH * ROPE_DIM], F32)
            nc.gpsimd.dma_start(bqrT_sb[:], bqrT[:])
            cos_sb = pc.tile([P, NT, NH, ROPE_DIM // 2], BF16)
            nc.gpsimd.dma_start(
                cos_sb[:], cosT.rearrange("p (t h i) -> p t h i", t=NT, h=NH)
            )
            sin_sb = pc.tile([P, NT, NH, ROPE_DIM // 2], BF16)
            nc.gpsimd.dma_start(
                sin_sb[:], sinT.rearrange("p (t h i) -> p t h i", t=NT, h=NH)
            )

            kvq_sb = pkvq.tile([P, NKT, S], BF16)

            # up-projection weights, loaded once, at t=0 (scalar DMA queue)
            p2w = tc.alloc_tile_pool(name="p2w", bufs=1)
            wku_t = p2w.tile([P, 4, NH * HEAD_DIM], BF16, tag="wku")
            nc.scalar.dma_start(wku_t[:], Wku.rearrange("(t p) m -> p t m", p=P))
            wvu_t = p2w.tile([P, 4, NH * HEAD_DIM], BF16, tag="wvu")
            nc.scalar.dma_start(wvu_t[:], Wvu.rearrange("(t p) m -> p t m", p=P))
            wqu_t = p2w.tile([P, 12, NH * HEAD_DIM], BF16, tag="wqu")
            nc.scalar.dma_start(wqu_t[:], Wqu.rearrange("(t p) m -> p t m", p=P))
            # rope weights kept token-major (they are the rhs)
            wkr_t = p2w.tile([P, 4, NH * ROPE_DIM], BF16, tag="wkr")
            nc.scalar.dma_start(wkr_t[:], Wkr.rearrange("(t p) m -> p t m", p=P))
            wqr_t = p2w.tile([P, 12, NH * ROPE_DIM], BF16, tag="wqr")
            nc.scalar.dma_start(wqr_t[:], Wqr.rearrange("(t p) m -> p t m", p=P))

            # ---------------- phase 1: down projection + AllGather --------
            with (
                tc.tile_pool(name="p1", bufs=1) as p1,
                tc.tile_pool(name="ps1", bufs=6, space="PSUM") as ps1,
            ):
                xt = p1.tile([P, NKT, NSC * P], BF16)
                for j in range(NSC):
                    nc.sync.dma_start(
                        xt[:, :, j * P : (j + 1) * P],
                        xTo[:, j * P : (j + 1) * P].rearrange(
                            "(t p) s -> p t s", p=P
                        ),
                    )
                wd_t = p1.tile([P, NKT, KV_C + Q_C], BF16)
                for q8 in range(8):
                    nc.gpsimd.dma_start(
                        wd_t[:, :, q8 * 256 : (q8 + 1) * 256],
                        Wd[:, q8 * 256 : (q8 + 1) * 256].rearrange(
                            "(t p) m -> p t m", p=P
                        ),
                    )
                kvq_own = p1.tile([P, NMD, NSC * P], BF16)
                for j in range(NSC):
                    for m in range(NMD):
                        ps = ps1.tile([P, P], F32, tag="mm")
                        for k in range(NKT):
                            nc.tensor.matmul(
                                ps[:],
                                wd_t[:, k, m * P : (m + 1) * P],
                                xt[:, k, j * P : (j + 1) * P],
                                start=(k == 0),
                                stop=(k == NKT - 1),
                            )
                        nc.vector.tensor_scalar_add(
                            kvq_own[:, m, j * P : (j + 1) * P],
                            ps[:],
                            bd_sb[:, m : m + 1],
                        )
                    nc.sync.dma_start(
                        cc_in[j][:], kvq_own[:, :, j * P : (j + 1) * P]
                    )
                for j in range(NSC):
                    nc.gpsimd.collective_compute(
                        "AllGather",
                        mybir.AluOpType.bypass,
                        replica_groups=GROUPS,
                        ins=[cc_in[j][:].opt()],
                        outs=[cc_out[j][:].opt()],
                    )
                for j in range(NSC):
                    for rr in range(4):
                        c0 = j * SC + rr * P
                        nc.sync.dma_start(
                            kvq_sb[:, :, c0 : c0 + P], cc_out[j][rr]
                        )

            # ------------- phases 2+3 interleaved per token chunk ---------
            with (
                tc.tile_pool(name="qkv", bufs=1) as pq,
                tc.tile_pool(name="p2t", bufs=2) as p2t,
                tc.tile_pool(name="pat", bufs=6) as pat,
                tc.tile_pool(name="patx", bufs=2) as patx,
                tc.tile_pool(name="patt", bufs=2) as patt,
                tc.tile_pool(name="pout", bufs=4) as pout,
                tc.tile_pool(name="pow", bufs=4) as pow_,
                tc.tile_pool(name="psA", bufs=2, space="PSUM") as psA,
                tc.tile_pool(name="psB", bufs=2, space="PSUM") as psB,
            ):
                kc_sb = pq.tile([P, NH, S], BF16)
                kr_sb = pq.tile([P, 2, S], BF16)
                qc_sb = pq.tile([P, NH, S], BF16)
                qr_sb = pq.tile([P, 2, S], BF16)
                v_sb = pq.tile([P, NT, NH * HEAD_DIM], BF16)

                def rope_tokmajor(tt, nk, koff, w_t, bias_sb, dst_sb):
                    """Rope for token tile tt (128 tokens): project with
                    tokens on partitions, rotate on DVE, transpose back."""
                    ps = psA.tile([P, NH * ROPE_DIM], F32, tag="mm")
                    for k in range(nk):
                        nc.tensor.matmul(
                            ps[:],
                            kvq_sb[:, koff + k, tt * P : (tt + 1) * P],
                            w_t[:, k, :],
                            start=(k == 0),
                            stop=(k == nk - 1),
                        )
                    pre = p2t.tile([P, NH * ROPE_DIM], F32, tag="ropeadd")
                    nc.vector.tensor_tensor(
                        pre[:], ps[:], bias_sb[:], mybir.AluOpType.add
                    )
                    prr = pre.rearrange("p (h i two) -> p h i two", h=NH, two=2)
                    rot = p2t.tile([P, NH, ROPE_DIM // 2, 2], BF16, tag="rot")
                    t1 = p2t.tile([P, NH, ROPE_DIM // 2], F32, tag="ropet1")
                    # even outputs: x1*cos - x2*sin
                    nc.vector.tensor_tensor(
                        t1[:], prr[:, :, :, 0], cos_sb[:, tt],
                        mybir.AluOpType.mult,
                    )
                    t2 = p2t.tile([P, NH, ROPE_DIM // 2], F32, tag="ropet2")
                    nc.vector.tensor_tensor(
                        t2[:], prr[:, :, :, 1], sin_sb[:, tt],
                        mybir.AluOpType.mult,
                    )
                    nc.vector.tensor_tensor(
                        rot[:, :, :, 0], t1[:], t2[:], mybir.AluOpType.subtract
                    )
                    # odd outputs: x1*sin + x2*cos
                    nc.vector.tensor_tensor(
                        t1[:], prr[:, :, :, 0], sin_sb[:, tt],
                        mybir.AluOpType.mult,
                    )
                    nc.vector.tensor_tensor(
                        t2[:], prr[:, :, :, 1], cos_sb[:, tt],
                        mybir.AluOpType.mult,
                    )
                    nc.vector.tensor_tensor(
                        rot[:, :, :, 1], t1[:], t2[:], mybir.AluOpType.add
                    )
                    # transpose [tok, 128] blocks back to feature-major
                    rotf = rot.rearrange("p h i two -> p (h i two)")
                    for hb in range(2):
                        pt = psA.tile([P, P], BF16, tag="mm")
                        nc.tensor.transpose(
                            pt[:], rotf[:, hb * P : (hb + 1) * P], eye_sb[:]
                        )
                        nc.scalar.copy(
                            dst_sb[:, hb, tt * P : (tt + 1) * P], pt[:]
                        )

                for qc in range(NSC):
                    qsl = slice(qc * SC, (qc + 1) * SC)
                    # ---- up-projections for this token chunk ----
                    for m in range(NH):
                        ps = psA.tile([P, SC], F32, tag="mm")
                        for k in range(4):
                            nc.tensor.matmul(
                                ps[:],
                                wku_t[:, k, m * P : (m + 1) * P],
                                kvq_sb[:, k, qsl],
                                start=(k == 0),
                                stop=(k == 3),
                            )
                        nc.vector.tensor_scalar_add(
                            kc_sb[:, m, qsl], ps[:], bku_sb[:, m : m + 1]
                        )
                    for t in range(4):
                        tt = qc * 4 + t
                        ps = psA.tile([P, NH * HEAD_DIM], F32, tag="mm")
                        for k in range(4):
                            nc.tensor.matmul(
                                ps[:],
                                kvq_sb[:, k, tt * P : (tt + 1) * P],
                                wvu_t[:, k, :],
                                start=(k == 0),
                                stop=(k == 3),
                            )
                        nc.vector.tensor_copy(v_sb[:, tt, :], ps[:])
                        rope_tokmajor(tt, 4, 0, wkr_t, bkrT_sb, kr_sb)
                    for m in range(NH):
                        ps = psA.tile([P, SC], F32, tag="mm")
                        for k in range(12):
                            nc.tensor.matmul(
                                ps[:],
                                wqu_t[:, k, m * P : (m + 1) * P],
                                kvq_sb[:, 4 + k, qsl],
                                start=(k == 0),
                                stop=(k == 11),
                            )
                        nc.vector.tensor_scalar_add(
                            qc_sb[:, m, qsl], ps[:], bqu_sb[:, m : m + 1]
                        )
                    for t in range(4):
                        rope_tokmajor(qc * 4 + t, 12, 4, wqr_t, bqrT_sb, qr_sb)

                    # ---- attention for this q-chunk ----
                    nkb = 4 * qc + 4
                    ctx_q = patx.tile([P, NH, SC], BF16, tag="ctx")
                    for h in range(NH):
                        hc = h // 2
                        hp = (h % 2) * ROPE_DIM
                        psum_ctx = psB.tile([P, SC], F32, tag="ctx")
                        acc = patt.tile([P, SC], F32, tag="acc")
                        for kb in range(nkb):
                            ksl = slice(kb * P, (kb + 1) * P)
                            diag = kb >= 4 * qc
                            c = (kb - 4 * qc) * P if diag else 0
                            qs0 = qc * SC + c
                            ps = psA.tile([P, SC], F32, tag="sc")
                            nc.tensor.matmul(
                                ps[:, c:],
                                kc_sb[:, h, ksl],
                                qc_sb[:, h, qs0 : (qc + 1) * SC],
                                start=True, stop=False,
                            )
                            nc.tensor.matmul(
                                ps[:, c:],
                                kr_sb[hp : hp + ROPE_DIM, hc, ksl],
                                qr_sb[hp : hp + ROPE_DIM, hc,
                                      qs0 : (qc + 1) * SC],
                                start=False, stop=True,
                            )
                            probs = pat.tile([P, SC], BF16, tag="probs")
                            if diag:
                                nc.vector.tensor_tensor(
                                    ps[:, c : c + P],
                                    ps[:, c : c + P],
                                    tri_sb[:],
                                    mybir.AluOpType.add,
                                )
                            nc.scalar.activation(
                                probs[:, c:], ps[:, c:], AF.Exp, scale=SCALE
                            )
                            if kb == 0:
                                nc.vector.tensor_copy(acc[:], probs[:])
                            else:
                                nc.vector.tensor_tensor(
                                    acc[:, c:], acc[:, c:], probs[:, c:],
                                    mybir.AluOpType.add,
                                )
                            nc.tensor.matmul(
                                psum_ctx[:, c:],
                                v_sb[:, kb, h * P : (h + 1) * P],
                                probs[:, c:],
                                start=(kb == 0), stop=(kb == nkb - 1),
                            )
                        rcp = patt.tile([P, SC], F32, tag="rcp")
                        if USE_PAR:
                            dsum = patt.tile([P, SC], F32, tag="dsum")
                            nc.gpsimd.partition_all_reduce(
                                dsum[:], acc[:], 128, bass_isa.ReduceOp.add
                            )
                            nc.vector.reciprocal(rcp[:], dsum[:])
                        else:
                            acc16 = patt.tile([P, SC], BF16, tag="acc16")
                            nc.vector.tensor_copy(acc16[:], acc[:])
                            psd = psB.tile([P, SC], F32, tag="m")
                            nc.tensor.matmul(
                                psd[:], ones_sb[:], acc16[:],
                                start=True, stop=True,
                            )
                            nc.vector.reciprocal(rcp[:], psd[:])
                        nc.vector.tensor_tensor(
                            ctx_q[:, h, :], psum_ctx[:], rcp[:],
                            mybir.AluOpType.mult,
                        )

                    # ---- out-projection for this q-chunk ----
                    for m in range(NMD):
                        wo_t = pow_.tile([P, NH, P], BF16, tag="wo")
                        nc.gpsimd.dma_start(
                            wo_t[:],
                            Wo[:, m * P : (m + 1) * P].rearrange(
                                "(t p) m -> p t m", p=P
                            ),
                        )
                        ps = psB.tile([P, SC], F32, tag="m")
                        for k in range(NH):
                            nc.tensor.matmul(
                                ps[:],
                                wo_t[:, k, :],
                                ctx_q[:, k, :],
                                start=(k == 0),
                                stop=(k == NH - 1),
                            )
                        og = pout.tile([P, SC], BF16, tag="og")
                        nc.vector.tensor_copy(og[:], ps[:])
                        nc.scalar.dma_start(
                            outT[m * P : (m + 1) * P, qsl], og[:]
                        )
            p2w.release()
    _split_waits(nc)
    return nc


def _col_bias(b, nm):
    """[nm*128] -> [128, nm] (column m = bias for feature chunk m)."""
    return np.ascontiguousarray(b.reshape(nm, P).T).astype(np.float32)


_NC = None


def kernel(**inputs):
    global _NC
    inp = {k: np.asarray(v) for k, v in inputs.items()}
    x = inp["x"].astype(np.float32)

    Wd_full = np.concatenate(
        [inp["kv_down_w"], inp["query_down_w"]], axis=1
    ).astype(BF)
    bd_full = np.concatenate([inp["kv_down_b"], inp["query_down_b"]])

    # token-major cos/sin: cosT[p, (t, h, i)] = cos(pos(t*128+p)*inv_freq[i])
    pos = np.arange(S, dtype=np.float64)
    inv = 1.0 / (10000.0 ** (np.arange(0, ROPE_DIM, 2, np.float64) / ROPE_DIM))
    ang = pos[:, None] * inv[None, :]              # [S, 32]
    cosS = np.cos(ang).reshape(NT, P, ROPE_DIM // 2)  # [t, p, i]
    sinS = np.sin(ang).reshape(NT, P, ROPE_DIM // 2)
    cosT = np.repeat(
        cosS.transpose(1, 0, 2)[:, :, None, :], NH, axis=2
    ).reshape(P, -1).astype(BF)
    sinT = np.repeat(
        sinS.transpose(1, 0, 2)[:, :, None, :], NH, axis=2
    ).reshape(P, -1).astype(BF)
    tri = np.where(
        np.arange(P)[None, :] >= np.arange(P)[:, None], 0.0, NEG
    ).astype(np.float32)
    eye = np.eye(P, dtype=np.float32).astype(BF)

    in_maps = []
    for c in range(8):
        b, r = c // 4, c % 4
        h0 = r * NH
        csl = slice(h0 * HEAD_DIM, (h0 + NH) * HEAD_DIM)
        rsl = slice(h0 * ROPE_DIM, (h0 + NH) * ROPE_DIM)
        own_cols = np.concatenate(
            [np.arange(SC * j + P * r, SC * j + P * r + P) for j in range(NSC)]
        )
        in_maps.append(
            {
                "xTo": np.ascontiguousarray(x[b].T[:, own_cols]).astype(BF),
                "Wd": Wd_full,
                "bd": _col_bias(bd_full, NMD),
                "Wku": inp["key_up_w"][:, csl].astype(BF),
                "bku": _col_bias(inp["key_up_b"][csl], 4),
                "Wvu": inp["value_up_w"][:, csl].astype(BF),
                "Wkr": inp["key_rope_w"][:, rsl].astype(BF),
                "Wqu": inp["query_up_w"][:, csl].astype(BF),
                "bqu": _col_bias(inp["query_up_b"][csl], 4),
                "Wqr": inp["query_rope_w"][:, rsl].astype(BF),
                "bkrT": np.broadcast_to(
                    inp["key_rope_b"][rsl].astype(np.float32),
                    (P, NH * ROPE_DIM),
                ).copy(),
                "bqrT": np.broadcast_to(
                    inp["query_rope_b"][rsl].astype(np.float32),
                    (P, NH * ROPE_DIM),
                ).copy(),
                "Wo": inp["out_w"][csl, :].astype(BF),
                "cosT": cosT,
                "sinT": sinT,
                "tri": tri,
                "eye": eye,
            }
        )

    if _NC is None:
        _NC = build()
    res = run_bass_kernel_spmd(_NC, in_maps, core_ids=list(range(8)))

    corr = (
        inp["value_up_b"].astype(np.float32) @ inp["out_w"].astype(np.float32)
        + inp["out_b"].astype(np.float32)
    )
    out = np.empty((B, S, HIDDEN), np.float32)
    for b in range(B):
        acc = res.results[b * 4]["outT"].astype(np.float32)
        for g in range(1, 4):
            acc += res.results[b * 4 + g]["outT"].astype(np.float32)
        out[b] = acc.T + corr[None, :]
    return out


# revision 36
# speedup vs baseline: 1.2139x; 1.1972x over previous
"""Multi-Head Latent Attention on 8 Trainium2 NeuronCores.

Sharding: core c = (batch b = c//4) x (head-group g = c%4, 4 heads each).

The kv down projection is token-sharded across the 4 cores of a batch
group: core (b, r) computes kv_c^T for token slices {512j + 128r : j} and
the full kv_c is assembled with four pipelined AllGathers (one per
512-token chunk, 0.5 MB each), overlapped with compute.  The query path
needs no collective at all: the host folds W_q = query_down_w @
query_up_w (and the rope analog) so each core computes its 4 heads'
queries directly from x^T — per-core FLOPs for the query side are lower
than sharing the 1536-dim latent, since only 768 output dims are needed.
Each core computes K/V/rope/attention for its 4 heads and a partial
output projection; the host sums the 4 bf16 partials per batch and adds
the output bias (plus the value-up bias folded through out_w, exact
because softmax rows sum to 1).

All layouts are feature-major: x^T, kv_c^T, K^T, Q^T, ctx^T, out^T.
Scores are computed as scores^T[k, q] so probs^T feeds the context
matmul directly.  Softmax denominators are accumulated on the vector
engine (bf16) and reduced across partitions with one ones-matmul per
(chunk, head).  Exp is applied without max-subtraction (scores for this
problem are in [-1, 1]).

Rope is applied token-major: projections computed once with tokens on
partitions (weights streaming), rotated pairwise on the vector engine,
transposed back to feature-major with PE transposes.
"""

import numpy as np
import ml_dtypes

import concourse.bass as bass
import concourse.mybir as mybir
from concourse.tile import TileContext
from concourse.bass_utils import run_bass_kernel_spmd

F32 = mybir.dt.float32
BF16 = mybir.dt.bfloat16
AF = mybir.ActivationFunctionType
BF = ml_dtypes.bfloat16

HIDDEN = 2048
NUM_HEADS = 16
HEAD_DIM = 128
KV_C = 512
Q_C = 1536
ROPE_DIM = 64
B, S = 2, 2048

P = 128
NH = 4          # heads per core
SC = 512        # token chunk (one AllGather / one attention q-chunk)
NKT = HIDDEN // P       # 16 k-tiles over x features
NKV = KV_C // P         # 4 kv_c feature tiles
NSC = S // SC           # 4 chunks
NT = S // P             # 16 token tiles
SCALE = float(1.0 / np.sqrt(HEAD_DIM + ROPE_DIM))
NEG = -1.0e5
GROUPS = [[0, 1, 2, 3], [4, 5, 6, 7]]


def _split_waits(nc, maxw=1):
    """This container's walrus accepts at most one sem-wait per instruction;
    move excess waits onto same-engine NOPs inserted immediately before."""
    for fn in nc.m.functions:
        for bb in fn.blocks:
            newlist = []
            for ins in bb.instructions:
                si = ins.sync_info
                if si is not None and si.on_wait is not None and len(si.on_wait) > maxw:
                    waits = list(si.on_wait)
                    extra, keep = waits[:-maxw], waits[-maxw:]
                    for k, i in enumerate(range(0, len(extra), maxw)):
                        nop = mybir.InstNoOp(
                            name=f"{ins.name}-waitsplit-{k}", ins=[], outs=[]
                        )
                        nop.engine = ins.engine
                        nop.sync_info = mybir.SyncInfo(
                            on_wait=extra[i : i + maxw], on_update=[]
                        )
                        newlist.append(nop)
                    ins.sync_info = mybir.SyncInfo(
                        on_wait=keep, on_update=list(si.on_update or [])
                    )
                newlist.append(ins)
            bb.instructions = newlist


def build():
    nc = bass.Bass(num_devices=8)
    dt = nc.dram_tensor
    xT = dt("xT", [HIDDEN, S], BF16, kind="ExternalInput")
    xTo = dt("xTo", [HIDDEN, NSC * P], BF16, kind="ExternalInput")
    Wdkv = dt("Wdkv", [HIDDEN, KV_C], BF16, kind="ExternalInput")
    bdkv = dt("bdkv", [P, NKV], F32, kind="ExternalInput")
    Wku = dt("Wku", [KV_C, NH * HEAD_DIM], BF16, kind="ExternalInput")
    bku = dt("bku", [P, 4], F32, kind="ExternalInput")
    Wvu = dt("Wvu", [KV_C, NH * HEAD_DIM], BF16, kind="ExternalInput")
    Wkr = dt("Wkr", [KV_C, NH * ROPE_DIM], BF16, kind="ExternalInput")
    Wq = dt("Wq", [HIDDEN, NH * HEAD_DIM], BF16, kind="ExternalInput")
    bq = dt("bq", [P, 4], F32, kind="ExternalInput")
    Wqr = dt("Wqr", [HIDDEN, NH * ROPE_DIM], BF16, kind="ExternalInput")
    # rope biases, token-major broadcast: [P, NH*ROPE_DIM] (same per token)
    bkrT = dt("bkrT", [P, NH * ROPE_DIM], F32, kind="ExternalInput")
    bqrT = dt("bqrT", [P, NH * ROPE_DIM], F32, kind="ExternalInput")
    Wo = dt("Wo", [NH * HEAD_DIM, HIDDEN], BF16, kind="ExternalInput")
    # token-major cos/sin: [P tokens-within-tile, tile, NH, ROPE_DIM/2]
    cosT = dt("cosT", [P, NT * NH * (ROPE_DIM // 2)], BF16, kind="ExternalInput")
    sinT = dt("sinT", [P, NT * NH * (ROPE_DIM // 2)], BF16, kind="ExternalInput")
    tri = dt("tri", [P, P], F32, kind="ExternalInput")
    eye = dt("eye", [P, P], BF16, kind="ExternalInput")
    outT = dt("outT", [HIDDEN, S], BF16, kind="ExternalOutput")

    cc_in = [dt(f"cc_in{j}", [P, NKV, P], BF16, kind="Internal")
             for j in range(NSC)]
    cc_out = [dt(f"cc_out{j}", [4, P, NKV, P], BF16, kind="Internal")
              for j in range(NSC)]

    with TileContext(nc) as tc:
        with (
            tc.tile_pool(name="const", bufs=1) as pc,
            tc.tile_pool(name="kv", bufs=1) as pkv,
            tc.tile_pool(name="xp", bufs=1) as pxp,
            tc.tile_pool(name="wts", bufs=1) as pw,
        ):
            # --- constants (gpsimd DMA queue) ---
            tri_sb = pc.tile([P, P], F32)
            nc.gpsimd.dma_start(tri_sb[:], tri[:])
            eye_sb = pc.tile([P, P], BF16)
            nc.gpsimd.dma_start(eye_sb[:], eye[:])
            ones_sb = pc.tile([P, P], BF16)
            nc.vector.memset(ones_sb[:], 1.0)
            bdkv_sb = pc.tile([P, NKV], F32)
            nc.gpsimd.dma_start(bdkv_sb[:], bdkv[:])
            bku_sb = pc.tile([P, 4], F32)
            nc.gpsimd.dma_start(bku_sb[:], bku[:])
            bq_sb = pc.tile([P, 4], F32)
            nc.gpsimd.dma_start(bq_sb[:], bq[:])
            bkrT_sb = pc.tile([P, NH * ROPE_DIM], F32)
            nc.gpsimd.dma_start(bkrT_sb[:], bkrT[:])
            bqrT_sb = pc.tile([P, NH * ROPE_DIM], F32)
            nc.gpsimd.dma_start(bqrT_sb[:], bqrT[:])
            cos_sb = pc.tile([P, NT, NH, ROPE_DIM // 2], BF16)
            nc.gpsimd.dma_start(
                cos_sb[:], cosT.rearrange("p (t h i) -> p t h i", t=NT, h=NH)
            )
            sin_sb = pc.tile([P, NT, NH, ROPE_DIM // 2], BF16)
            nc.gpsimd.dma_start(
                sin_sb[:], sinT.rearrange("p (t h i) -> p t h i", t=NT, h=NH)
            )

            kv_sb = pkv.tile([P, NKV, S], BF16)

            def load_xt_chunk(qc):
                t = pxp.tile([P, NKT, SC], BF16, tag="xt", bufs=2)
                nc.scalar.dma_start(
                    t[:],
                    xT[:, qc * SC : (qc + 1) * SC].rearrange(
                        "(t p) s -> p t s", p=P
                    ),
                )
                return t

            # kv down-projection weights first (gpsimd queue, 2 MB)
            wdkv_t = pw.tile([P, NKT, KV_C], BF16, tag="wdkv")
            for q4 in range(4):
                nc.gpsimd.dma_start(
                    wdkv_t[:, :, q4 * P : (q4 + 1) * P],
                    Wdkv[:, q4 * P : (q4 + 1) * P].rearrange(
                        "(t p) m -> p t m", p=P
                    ),
                )
            # remaining weights (scalar queue; query-side first — needed
            # before the first AllGather lands, key-side after)
            wq_t = pw.tile([P, NKT, NH * HEAD_DIM], BF16, tag="wq")
            nc.scalar.dma_start(wq_t[:], Wq.rearrange("(t p) m -> p t m", p=P))
            wqr_t = pw.tile([P, NKT, NH * ROPE_DIM], BF16, tag="wqr")
            nc.scalar.dma_start(wqr_t[:], Wqr.rearrange("(t p) m -> p t m", p=P))
            wku_t = pw.tile([P, NKV, NH * HEAD_DIM], BF16, tag="wku")
            nc.scalar.dma_start(wku_t[:], Wku.rearrange("(t p) m -> p t m", p=P))
            wvu_t = pw.tile([P, NKV, NH * HEAD_DIM], BF16, tag="wvu")
            nc.scalar.dma_start(wvu_t[:], Wvu.rearrange("(t p) m -> p t m", p=P))
            wkr_t = pw.tile([P, NKV, NH * ROPE_DIM], BF16, tag="wkr")
            nc.scalar.dma_start(wkr_t[:], Wkr.rearrange("(t p) m -> p t m", p=P))

            # ---------------- phase 1: kv down projection + AllGather -----
            with (
                tc.tile_pool(name="p1", bufs=1) as p1,
                tc.tile_pool(name="ps1", bufs=6, space="PSUM") as ps1,
            ):
                # own token columns for the kv down projection (host-packed)
                xto = p1.tile([P, NKT, NSC * P], BF16)
                for j in range(NSC):
                    nc.sync.dma_start(
                        xto[:, :, j * P : (j + 1) * P],
                        xTo[:, j * P : (j + 1) * P].rearrange(
                            "(t p) s -> p t s", p=P
                        ),
                    )
                kv_own = p1.tile([P, NKV, NSC * P], BF16)
                for j in range(NSC):
                    for m in range(NKV):
                        ps = ps1.tile([P, P], F32, tag="mm")
                        for k in range(NKT):
                            nc.tensor.matmul(
                                ps[:],
                                wdkv_t[:, k, m * P : (m + 1) * P],
                                xto[:, k, j * P : (j + 1) * P],
                                start=(k == 0),
                                stop=(k == NKT - 1),
                            )
                        nc.vector.tensor_scalar_add(
                            kv_own[:, m, j * P : (j + 1) * P],
                            ps[:],
                            bdkv_sb[:, m : m + 1],
                        )
                    nc.sync.dma_start(
                        cc_in[j][:], kv_own[:, :, j * P : (j + 1) * P]
                    )
                for j in range(NSC):
                    nc.gpsimd.collective_compute(
                        "AllGather",
                        mybir.AluOpType.bypass,
                        replica_groups=GROUPS,
                        ins=[cc_in[j][:].opt()],
                        outs=[cc_out[j][:].opt()],
                    )
                for j in range(NSC):
                    for rr in range(4):
                        c0 = j * SC + rr * P
                        nc.sync.dma_start(
                            kv_sb[:, :, c0 : c0 + P], cc_out[j][rr]
                        )

            # ------------- phases 2+3 interleaved per token chunk ---------
            with (
                tc.tile_pool(name="qkv", bufs=1) as pq,
                tc.tile_pool(name="p2t", bufs=2) as p2t,
                tc.tile_pool(name="pat", bufs=6) as pat,
                tc.tile_pool(name="patx", bufs=2) as patx,
                tc.tile_pool(name="patt", bufs=2) as patt,
                tc.tile_pool(name="pout", bufs=4) as pout,
                tc.tile_pool(name="pow", bufs=4) as pow_,
                tc.tile_pool(name="psA", bufs=2, space="PSUM") as psA,
                tc.tile_pool(name="psB", bufs=2, space="PSUM") as psB,
            ):
                kc_sb = pq.tile([P, NH, S], BF16)
                kr_sb = pq.tile([P, 2, S], BF16)
                v_sb = pq.tile([P, NT, NH * HEAD_DIM], BF16)

                def rope_tokmajor(tt, nk, src, src_t, w_t, bias_sb,
                                  dst_sb, dst_t):
                    """Rope for global token tile tt: project with tokens on
                    partitions (reading src tile src_t), rotate on DVE,
                    transpose back into dst tile dst_t."""
                    ps = psA.tile([P, NH * ROPE_DIM], F32, tag="mm")
                    for k in range(nk):
                        nc.tensor.matmul(
                            ps[:],
                            src[:, k, src_t * P : (src_t + 1) * P],
                            w_t[:, k, :],
                            start=(k == 0),
                            stop=(k == nk - 1),
                        )
                    pre = p2t.tile([P, NH * ROPE_DIM], BF16, tag="ropeadd")
                    nc.vector.tensor_tensor(
                        pre[:], ps[:], bias_sb[:], mybir.AluOpType.add
                    )
                    prr = pre.rearrange("p (h i two) -> p h i two", h=NH, two=2)
                    rot = p2t.tile([P, NH, ROPE_DIM // 2, 2], BF16, tag="rot")
                    t1 = p2t.tile([P, NH, ROPE_DIM // 2], BF16, tag="ropet1")
                    # even outputs: x1*cos - x2*sin
                    nc.vector.tensor_tensor(
                        t1[:], prr[:, :, :, 0], cos_sb[:, tt],
                        mybir.AluOpType.mult,
                    )
                    t2 = p2t.tile([P, NH, ROPE_DIM // 2], BF16, tag="ropet2")
                    nc.vector.tensor_tensor(
                        t2[:], prr[:, :, :, 1], sin_sb[:, tt],
                        mybir.AluOpType.mult,
                    )
                    nc.vector.tensor_tensor(
                        rot[:, :, :, 0], t1[:], t2[:], mybir.AluOpType.subtract
                    )
                    # odd outputs: x1*sin + x2*cos
                    nc.vector.tensor_tensor(
                        t1[:], prr[:, :, :, 0], sin_sb[:, tt],
                        mybir.AluOpType.mult,
                    )
                    nc.vector.tensor_tensor(
                        t2[:], prr[:, :, :, 1], cos_sb[:, tt],
                        mybir.AluOpType.mult,
                    )
                    nc.vector.tensor_tensor(
                        rot[:, :, :, 1], t1[:], t2[:], mybir.AluOpType.add
                    )
                    # transpose [tok, 128] blocks back to feature-major
                    rotf = rot.rearrange("p h i two -> p (h i two)")
                    for hb in range(2):
                        pt = psA.tile([P, P], BF16, tag="mm")
                        nc.tensor.transpose(
                            pt[:], rotf[:, hb * P : (hb + 1) * P], eye_sb[:]
                        )
                        nc.scalar.copy(
                            dst_sb[:, hb, dst_t * P : (dst_t + 1) * P], pt[:]
                        )

                xt_cur = load_xt_chunk(0)
                for qc in range(NSC):
                    qsl = slice(qc * SC, (qc + 1) * SC)
                    xt = xt_cur
                    if qc + 1 < NSC:
                        xt_cur = load_xt_chunk(qc + 1)
                    # ---- queries for this chunk: direct from x, no AG dep
                    qc_sb = pq.tile([P, NH, SC], BF16, tag="qc", bufs=2)
                    qr_sb = pq.tile([P, 2, SC], BF16, tag="qr", bufs=2)
                    for m in range(NH):
                        ps = psA.tile([P, SC], F32, tag="mm")
                        for k in range(NKT):
                            nc.tensor.matmul(
                                ps[:],
                                wq_t[:, k, m * P : (m + 1) * P],
                                xt[:, k, :],
                                start=(k == 0),
                                stop=(k == NKT - 1),
                            )
                        nc.vector.tensor_scalar_add(
                            qc_sb[:, m, :], ps[:], bq_sb[:, m : m + 1]
                        )
                    for t in range(4):
                        rope_tokmajor(qc * 4 + t, NKT, xt, t, wqr_t,
                                      bqrT_sb, qr_sb, t)

                    # ---- keys/values for this chunk (needs AllGather qc)
                    for m in range(NH):
                        ps = psA.tile([P, SC], F32, tag="mm")
                        for k in range(NKV):
                            nc.tensor.matmul(
                                ps[:],
                                wku_t[:, k, m * P : (m + 1) * P],
                                kv_sb[:, k, qsl],
                                start=(k == 0),
                                stop=(k == NKV - 1),
                            )
                        nc.vector.tensor_scalar_add(
                            kc_sb[:, m, qsl], ps[:], bku_sb[:, m : m + 1]
                        )
                    for t in range(4):
                        tt = qc * 4 + t
                        ps = psA.tile([P, NH * HEAD_DIM], F32, tag="mm")
                        for k in range(NKV):
                            nc.tensor.matmul(
                                ps[:],
                                kv_sb[:, k, tt * P : (tt + 1) * P],
                                wvu_t[:, k, :],
                                start=(k == 0),
                                stop=(k == NKV - 1),
                            )
                        nc.vector.tensor_copy(v_sb[:, tt, :], ps[:])
                        rope_tokmajor(tt, NKV, kv_sb, tt, wkr_t,
                                      bkrT_sb, kr_sb, tt)

                    # ---- attention for this q-chunk ----
                    nkb = 4 * qc + 4
                    ctx_q = patx.tile([P, NH, SC], BF16, tag="ctx")
                    for h in range(NH):
                        hc = h // 2
                        hp = (h % 2) * ROPE_DIM
                        psum_ctx = psB.tile([P, SC], F32, tag="ctx")
                        acc = patt.tile([P, SC], BF16, tag="acc")
                        for kb in range(nkb):
                            ksl = slice(kb * P, (kb + 1) * P)
                            diag = kb >= 4 * qc
                            c = (kb - 4 * qc) * P if diag else 0
                            qs0 = qc * SC + c
                            ps = psA.tile([P, SC], F32, tag="sc")
                            nc.tensor.matmul(
                                ps[:, c:],
                                kc_sb[:, h, ksl],
                                qc_sb[:, h, c:],
                                start=True, stop=False,
                            )
                            nc.tensor.matmul(
                                ps[:, c:],
                                kr_sb[hp : hp + ROPE_DIM, hc, ksl],
                                qr_sb[hp : hp + ROPE_DIM, hc, c:],
                                start=False, stop=True,
                            )
                            probs = pat.tile([P, SC], BF16, tag="probs")
                            if diag:
                                nc.vector.tensor_tensor(
                                    ps[:, c : c + P],
                                    ps[:, c : c + P],
                                    tri_sb[:],
                                    mybir.AluOpType.add,
                                )
                            nc.scalar.activation(
                                probs[:, c:], ps[:, c:], AF.Exp, scale=SCALE
                            )
                            if kb == 0:
                                nc.vector.tensor_copy(acc[:], probs[:])
                            else:
                                nc.vector.tensor_tensor(
                                    acc[:, c:], acc[:, c:], probs[:, c:],
                                    mybir.AluOpType.add,
                                )
                            nc.tensor.matmul(
                                psum_ctx[:, c:],
                                v_sb[:, kb, h * P : (h + 1) * P],
                                probs[:, c:],
                                start=(kb == 0), stop=(kb == nkb - 1),
                            )
                        psd = psB.tile([P, SC], F32, tag="m")
                        nc.tensor.matmul(
                            psd[:], ones_sb[:], acc[:], start=True, stop=True
                        )
                        rcp = patt.tile([P, SC], F32, tag="rcp")
                        nc.vector.reciprocal(rcp[:], psd[:])
                        nc.vector.tensor_tensor(
                            ctx_q[:, h, :], psum_ctx[:], rcp[:],
                            mybir.AluOpType.mult,
                        )

                    # ---- out-projection for this q-chunk ----
                    for m in range(NKT):
                        wo_t = pow_.tile([P, NH, P], BF16, tag="wo")
                        nc.gpsimd.dma_start(
                            wo_t[:],
                            Wo[:, m * P : (m + 1) * P].rearrange(
                                "(t p) m -> p t m", p=P
                            ),
                        )
                        ps = psB.tile([P, SC], F32, tag="m")
                        for k in range(NH):
                            nc.tensor.matmul(
                                ps[:],
                                wo_t[:, k, :],
                                ctx_q[:, k, :],
                                start=(k == 0),
                                stop=(k == NH - 1),
                            )
                        og = pout.tile([P, SC], BF16, tag="og")
                        nc.scalar.copy(og[:], ps[:])
                        nc.sync.dma_start(
                            outT[m * P : (m + 1) * P, qsl], og[:]
                        )
    _split_waits(nc)
    return nc


def _col_bias(b, nm):
    """[nm*128] -> [128, nm] (column m = bias for feature chunk m)."""
    return np.ascontiguousarray(b.reshape(nm, P).T).astype(np.float32)


_NC = None


def kernel(**inputs):
    global _NC
    inp = {k: np.asarray(v) for k, v in inputs.items()}
    x = inp["x"].astype(np.float32)

    # token-major cos/sin: cosT[p, (t, h, i)] = cos(pos(t*128+p)*inv_freq[i])
    pos = np.arange(S, dtype=np.float64)
    inv = 1.0 / (10000.0 ** (np.arange(0, ROPE_DIM, 2, np.float64) / ROPE_DIM))
    ang = pos[:, None] * inv[None, :]              # [S, 32]
    cosS = np.cos(ang).reshape(NT, P, ROPE_DIM // 2)  # [t, p, i]
    sinS = np.sin(ang).reshape(NT, P, ROPE_DIM // 2)
    cosT = np.repeat(
        cosS.transpose(1, 0, 2)[:, :, None, :], NH, axis=2
    ).reshape(P, -1).astype(BF)
    sinT = np.repeat(
        sinS.transpose(1, 0, 2)[:, :, None, :], NH, axis=2
    ).reshape(P, -1).astype(BF)
    tri = np.where(
        np.arange(P)[None, :] >= np.arange(P)[:, None], 0.0, NEG
    ).astype(np.float32)
    eye = np.eye(P, dtype=np.float32).astype(BF)

    qdw = inp["query_down_w"].astype(np.float32)
    qdb = inp["query_down_b"].astype(np.float32)

    in_maps = []
    for c in range(8):
        b, r = c // 4, c % 4
        h0 = r * NH
        csl = slice(h0 * HEAD_DIM, (h0 + NH) * HEAD_DIM)
        rsl = slice(h0 * ROPE_DIM, (h0 + NH) * ROPE_DIM)
        # fold the query path: Q = x @ (qdw @ qu) + (qdb @ qu + qub)
        wq = qdw @ inp["query_up_w"][:, csl].astype(np.float32)
        bq_f = qdb @ inp["query_up_w"][:, csl].astype(np.float32) \
            + inp["query_up_b"][csl].astype(np.float32)
        wqr = qdw @ inp["query_rope_w"][:, rsl].astype(np.float32)
        bqr_f = qdb @ inp["query_rope_w"][:, rsl].astype(np.float32) \
            + inp["query_rope_b"][rsl].astype(np.float32)
        own_cols = np.concatenate(
            [np.arange(SC * j + P * r, SC * j + P * r + P) for j in range(NSC)]
        )
        in_maps.append(
            {
                "xT": np.ascontiguousarray(x[b].T).astype(BF),
                "xTo": np.ascontiguousarray(x[b].T[:, own_cols]).astype(BF),
                "Wdkv": inp["kv_down_w"].astype(BF),
                "bdkv": _col_bias(inp["kv_down_b"], NKV),
                "Wku": inp["key_up_w"][:, csl].astype(BF),
                "bku": _col_bias(inp["key_up_b"][csl], 4),
                "Wvu": inp["value_up_w"][:, csl].astype(BF),
                "Wkr": inp["key_rope_w"][:, rsl].astype(BF),
                "Wq": wq.astype(BF),
                "bq": _col_bias(bq_f, 4),
                "Wqr": wqr.astype(BF),
                "bkrT": np.broadcast_to(
                    inp["key_rope_b"][rsl].astype(np.float32),
                    (P, NH * ROPE_DIM),
                ).copy(),
                "bqrT": np.broadcast_to(
                    bqr_f, (P, NH * ROPE_DIM)
                ).copy(),
                "Wo": inp["out_w"][csl, :].astype(BF),
                "cosT": cosT,
                "sinT": sinT,
                "tri": tri,
                "eye": eye,
            }
        )

    if _NC is None:
        _NC = build()
    res = run_bass_kernel_spmd(_NC, in_maps, core_ids=list(range(8)))

    corr = (
        inp["value_up_b"].astype(np.float32) @ inp["out_w"].astype(np.float32)
        + inp["out_b"].astype(np.float32)
    )
    out = np.empty((B, S, HIDDEN), np.float32)
    for b in range(B):
        acc = res.results[b * 4]["outT"].astype(np.float32)
        for g in range(1, 4):
            acc += res.results[b * 4 + g]["outT"].astype(np.float32)
        out[b] = acc.T + corr[None, :]
    return out


# revision 38
# speedup vs baseline: 1.3384x; 1.1026x over previous
"""Multi-Head Latent Attention on 8 Trainium2 NeuronCores.

Sharding: core c = (batch b = c//4) x (head-group g = c%4, 4 heads each).

The kv down projection is token-sharded across the 4 cores of a batch
group: core (b, r) computes kv_c^T for token slices {512j + 128r : j} and
the full kv_c is assembled with four pipelined AllGathers (one per
512-token chunk, 0.5 MB each), overlapped with compute.  The query path
needs no collective at all: the host folds W_q = query_down_w @
query_up_w (and the rope analog) so each core computes its 4 heads'
queries directly from x^T — per-core FLOPs for the query side are lower
than sharing the 1536-dim latent, since only 768 output dims are needed.
Each core computes K/V/rope/attention for its 4 heads and a partial
output projection; the host sums the 4 bf16 partials per batch and adds
the output bias (plus the value-up bias folded through out_w, exact
because softmax rows sum to 1).

All layouts are feature-major: x^T, kv_c^T, K^T, Q^T, ctx^T, out^T.
Scores are computed as scores^T[k, q] so probs^T feeds the context
matmul directly.  Softmax denominators are accumulated on the vector
engine (bf16) and reduced across partitions with one ones-matmul per
(chunk, head).  Exp is applied without max-subtraction (scores for this
problem are in [-1, 1]).

Rope is applied token-major: projections computed once with tokens on
partitions (weights streaming), rotated pairwise on the vector engine,
transposed back to feature-major with PE transposes.
"""

import numpy as np
import ml_dtypes

import concourse.bass as bass
import concourse.mybir as mybir
from concourse.tile import TileContext
from concourse.bass_utils import run_bass_kernel_spmd

F32 = mybir.dt.float32
BF16 = mybir.dt.bfloat16
AF = mybir.ActivationFunctionType
BF = ml_dtypes.bfloat16

HIDDEN = 2048
NUM_HEADS = 16
HEAD_DIM = 128
KV_C = 512
Q_C = 1536
ROPE_DIM = 64
B, S = 2, 2048

P = 128
NH = 4          # heads per core
SC = 512        # token chunk (one AllGather / one attention q-chunk)
NKT = HIDDEN // P       # 16 k-tiles over x features
NKV = KV_C // P         # 4 kv_c feature tiles
NSC = S // SC           # 4 chunks
NT = S // P             # 16 token tiles
SCALE = float(1.0 / np.sqrt(HEAD_DIM + ROPE_DIM))
NEG = -1.0e5
GROUPS = [[0, 1, 2, 3], [4, 5, 6, 7]]


def _split_waits(nc, maxw=1):
    """This container's walrus accepts at most one sem-wait per instruction;
    move excess waits onto same-engine NOPs inserted immediately before."""
    for fn in nc.m.functions:
        for bb in fn.blocks:
            newlist = []
            for ins in bb.instructions:
                si = ins.sync_info
                if si is not None and si.on_wait is not None and len(si.on_wait) > maxw:
                    waits = list(si.on_wait)
                    extra, keep = waits[:-maxw], waits[-maxw:]
                    for k, i in enumerate(range(0, len(extra), maxw)):
                        nop = mybir.InstNoOp(
                            name=f"{ins.name}-waitsplit-{k}", ins=[], outs=[]
                        )
                        nop.engine = ins.engine
                        nop.sync_info = mybir.SyncInfo(
                            on_wait=extra[i : i + maxw], on_update=[]
                        )
                        newlist.append(nop)
                    ins.sync_info = mybir.SyncInfo(
                        on_wait=keep, on_update=list(si.on_update or [])
                    )
                newlist.append(ins)
            bb.instructions = newlist


def build():
    nc = bass.Bass(num_devices=8)
    dt = nc.dram_tensor
    xT = dt("xT", [HIDDEN, S], BF16, kind="ExternalInput")
    xTo = dt("xTo", [HIDDEN, NSC * P], BF16, kind="ExternalInput")
    Wdkv = dt("Wdkv", [HIDDEN, KV_C], BF16, kind="ExternalInput")
    bdkv = dt("bdkv", [P, NKV], F32, kind="ExternalInput")
    Wku = dt("Wku", [KV_C, NH * HEAD_DIM], BF16, kind="ExternalInput")
    bku = dt("bku", [P, 4], F32, kind="ExternalInput")
    Wvu = dt("Wvu", [KV_C, NH * HEAD_DIM], BF16, kind="ExternalInput")
    Wkr = dt("Wkr", [KV_C, NH * ROPE_DIM], BF16, kind="ExternalInput")
    Wq = dt("Wq", [HIDDEN, NH * HEAD_DIM], BF16, kind="ExternalInput")
    bq = dt("bq", [P, 4], F32, kind="ExternalInput")
    Wqr = dt("Wqr", [HIDDEN, NH * ROPE_DIM], BF16, kind="ExternalInput")
    # rope biases, token-major broadcast: [P, NH*ROPE_DIM] (same per token)
    bkrT = dt("bkrT", [P, NH * ROPE_DIM], F32, kind="ExternalInput")
    bqrT = dt("bqrT", [P, NH * ROPE_DIM], F32, kind="ExternalInput")
    Wo = dt("Wo", [NH * HEAD_DIM, HIDDEN], BF16, kind="ExternalInput")
    # token-major cos/sin: [P tokens-within-tile, tile, NH, ROPE_DIM/2]
    cosT = dt("cosT", [P, NT * NH * (ROPE_DIM // 2)], BF16, kind="ExternalInput")
    sinT = dt("sinT", [P, NT * NH * (ROPE_DIM // 2)], BF16, kind="ExternalInput")
    tri = dt("tri", [P, P], F32, kind="ExternalInput")
    eye = dt("eye", [P, P], BF16, kind="ExternalInput")
    outT = dt("outT", [HIDDEN, S], BF16, kind="ExternalOutput")

    cc_in = [dt(f"cc_in{j}", [P, NKV, P], BF16, kind="Internal")
             for j in range(NSC)]
    cc_out = [dt(f"cc_out{j}", [4, P, NKV, P], BF16, kind="Internal")
              for j in range(NSC)]

    with TileContext(nc) as tc:
        with (
            tc.tile_pool(name="const", bufs=1) as pc,
            tc.tile_pool(name="kv", bufs=1) as pkv,
            tc.tile_pool(name="xp", bufs=1) as pxp,
            tc.tile_pool(name="wts", bufs=1) as pw,
        ):
            # --- constants (gpsimd DMA queue) ---
            tri_sb = pc.tile([P, P], F32)
            nc.gpsimd.dma_start(tri_sb[:], tri[:])
            eye_sb = pc.tile([P, P], BF16)
            nc.gpsimd.dma_start(eye_sb[:], eye[:])
            ones_sb = pc.tile([P, P], BF16)
            nc.vector.memset(ones_sb[:], 1.0)
            bdkv_sb = pc.tile([P, NKV], F32)
            nc.gpsimd.dma_start(bdkv_sb[:], bdkv[:])
            bku_sb = pc.tile([P, 4], F32)
            nc.gpsimd.dma_start(bku_sb[:], bku[:])
            bq_sb = pc.tile([P, 4], F32)
            nc.gpsimd.dma_start(bq_sb[:], bq[:])
            bkrT_sb = pc.tile([P, NH * ROPE_DIM], F32)
            nc.gpsimd.dma_start(bkrT_sb[:], bkrT[:])
            bqrT_sb = pc.tile([P, NH * ROPE_DIM], F32)
            nc.gpsimd.dma_start(bqrT_sb[:], bqrT[:])
            cos_sb = pc.tile([P, NT, NH, ROPE_DIM // 2], BF16)
            nc.gpsimd.dma_start(
                cos_sb[:], cosT.rearrange("p (t h i) -> p t h i", t=NT, h=NH)
            )
            sin_sb = pc.tile([P, NT, NH, ROPE_DIM // 2], BF16)
            nc.gpsimd.dma_start(
                sin_sb[:], sinT.rearrange("p (t h i) -> p t h i", t=NT, h=NH)
            )

            kv_sb = pkv.tile([P, NKV, S], BF16)

            def load_xt_chunk(qc):
                t = pxp.tile([P, NKT, SC], BF16, tag="xt", bufs=2)
                nc.scalar.dma_start(
                    t[:],
                    xT[:, qc * SC : (qc + 1) * SC].rearrange(
                        "(t p) s -> p t s", p=P
                    ),
                )
                return t

            # kv down-projection weights first (gpsimd queue, 2 MB)
            wdkv_t = pw.tile([P, NKT, KV_C], BF16, tag="wdkv")
            for q4 in range(4):
                nc.gpsimd.dma_start(
                    wdkv_t[:, :, q4 * P : (q4 + 1) * P],
                    Wdkv[:, q4 * P : (q4 + 1) * P].rearrange(
                        "(t p) m -> p t m", p=P
                    ),
                )
            # remaining weights (scalar queue; query-side first — needed
            # before the first AllGather lands, key-side after)
            wq_t = pw.tile([P, NKT, NH * HEAD_DIM], BF16, tag="wq")
            nc.scalar.dma_start(wq_t[:], Wq.rearrange("(t p) m -> p t m", p=P))
            wqr_t = pw.tile([P, NKT, NH * ROPE_DIM], BF16, tag="wqr")
            nc.scalar.dma_start(wqr_t[:], Wqr.rearrange("(t p) m -> p t m", p=P))
            wku_t = pw.tile([P, NKV, NH * HEAD_DIM], BF16, tag="wku")
            nc.scalar.dma_start(wku_t[:], Wku.rearrange("(t p) m -> p t m", p=P))
            wvu_t = pw.tile([P, NKV, NH * HEAD_DIM], BF16, tag="wvu")
            nc.scalar.dma_start(wvu_t[:], Wvu.rearrange("(t p) m -> p t m", p=P))
            wkr_t = pw.tile([P, NKV, NH * ROPE_DIM], BF16, tag="wkr")
            nc.scalar.dma_start(wkr_t[:], Wkr.rearrange("(t p) m -> p t m", p=P))

            # ---------------- phase 1: kv down projection + AllGather -----
            with (
                tc.tile_pool(name="p1", bufs=1) as p1,
                tc.tile_pool(name="ps1", bufs=6, space="PSUM") as ps1,
            ):
                # own token columns for the kv down projection (host-packed)
                xto = p1.tile([P, NKT, NSC * P], BF16)
                for j in range(NSC):
                    nc.sync.dma_start(
                        xto[:, :, j * P : (j + 1) * P],
                        xTo[:, j * P : (j + 1) * P].rearrange(
                            "(t p) s -> p t s", p=P
                        ),
                    )
                kv_own = p1.tile([P, NKV, NSC * P], BF16)
                for j in range(NSC):
                    for m in range(NKV):
                        ps = ps1.tile([P, P], F32, tag="mm")
                        for k in range(NKT):
                            nc.tensor.matmul(
                                ps[:],
                                wdkv_t[:, k, m * P : (m + 1) * P],
                                xto[:, k, j * P : (j + 1) * P],
                                start=(k == 0),
                                stop=(k == NKT - 1),
                            )
                        nc.vector.tensor_scalar_add(
                            kv_own[:, m, j * P : (j + 1) * P],
                            ps[:],
                            bdkv_sb[:, m : m + 1],
                        )
                    nc.sync.dma_start(
                        cc_in[j][:], kv_own[:, :, j * P : (j + 1) * P]
                    )
                for j in range(NSC):
                    nc.gpsimd.collective_compute(
                        "AllGather",
                        mybir.AluOpType.bypass,
                        replica_groups=GROUPS,
                        ins=[cc_in[j][:].opt()],
                        outs=[cc_out[j][:].opt()],
                    )
                for j in range(NSC):
                    for rr in range(4):
                        c0 = j * SC + rr * P
                        nc.sync.dma_start(
                            kv_sb[:, :, c0 : c0 + P], cc_out[j][rr]
                        )

            # ------------- phases 2+3 interleaved per token chunk ---------
            with (
                tc.tile_pool(name="qkv", bufs=1) as pq,
                tc.tile_pool(name="p2t", bufs=2) as p2t,
                tc.tile_pool(name="pat", bufs=6) as pat,
                tc.tile_pool(name="patx", bufs=2) as patx,
                tc.tile_pool(name="patt", bufs=2) as patt,
                tc.tile_pool(name="pout", bufs=4) as pout,
                tc.tile_pool(name="pow", bufs=4) as pow_,
                tc.tile_pool(name="psA", bufs=2, space="PSUM") as psA,
                tc.tile_pool(name="psB", bufs=2, space="PSUM") as psB,
            ):
                kc_sb = pq.tile([P, NH, S], BF16)
                kr_sb = pq.tile([P, 2, S], BF16)
                v_sb = pq.tile([P, NT, NH * HEAD_DIM], BF16)

                def rope_tokmajor(tt, nk, src, src_t, w_t, bias_sb,
                                  dst_sb, dst_t):
                    """Rope for global token tile tt: project with tokens on
                    partitions (reading src tile src_t), rotate on DVE,
                    transpose back into dst tile dst_t."""
                    ps = psA.tile([P, NH * ROPE_DIM], F32, tag="mm")
                    for k in range(nk):
                        nc.tensor.matmul(
                            ps[:],
                            src[:, k, src_t * P : (src_t + 1) * P],
                            w_t[:, k, :],
                            start=(k == 0),
                            stop=(k == nk - 1),
                        )
                    pre = p2t.tile([P, NH * ROPE_DIM], BF16, tag="ropeadd")
                    nc.vector.tensor_tensor(
                        pre[:], ps[:], bias_sb[:], mybir.AluOpType.add
                    )
                    prr = pre.rearrange("p (h i two) -> p h i two", h=NH, two=2)
                    rot = p2t.tile([P, NH, ROPE_DIM // 2, 2], BF16, tag="rot")
                    t1 = p2t.tile([P, NH, ROPE_DIM // 2], BF16, tag="ropet1")
                    # even outputs: x1*cos - x2*sin
                    nc.vector.tensor_tensor(
                        t1[:], prr[:, :, :, 0], cos_sb[:, tt],
                        mybir.AluOpType.mult,
                    )
                    t2 = p2t.tile([P, NH, ROPE_DIM // 2], BF16, tag="ropet2")
                    nc.vector.tensor_tensor(
                        t2[:], prr[:, :, :, 1], sin_sb[:, tt],
                        mybir.AluOpType.mult,
                    )
                    nc.vector.tensor_tensor(
                        rot[:, :, :, 0], t1[:], t2[:], mybir.AluOpType.subtract
                    )
                    # odd outputs: x1*sin + x2*cos
                    nc.vector.tensor_tensor(
                        t1[:], prr[:, :, :, 0], sin_sb[:, tt],
                        mybir.AluOpType.mult,
                    )
                    nc.vector.tensor_tensor(
                        t2[:], prr[:, :, :, 1], cos_sb[:, tt],
                        mybir.AluOpType.mult,
                    )
                    nc.vector.tensor_tensor(
                        rot[:, :, :, 1], t1[:], t2[:], mybir.AluOpType.add
                    )
                    # transpose [tok, 128] blocks back to feature-major
                    rotf = rot.rearrange("p h i two -> p (h i two)")
                    for hb in range(2):
                        pt = psA.tile([P, P], BF16, tag="mm")
                        nc.tensor.transpose(
                            pt[:], rotf[:, hb * P : (hb + 1) * P], eye_sb[:]
                        )
                        nc.scalar.copy(
                            dst_sb[:, hb, dst_t * P : (dst_t + 1) * P], pt[:]
                        )

                xt_cur = load_xt_chunk(0)
                q_tiles = []
                for qc in range(NSC):
                    qsl = slice(qc * SC, (qc + 1) * SC)
                    xt = xt_cur
                    if qc + 1 < NSC:
                        xt_cur = load_xt_chunk(qc + 1)
                    # ---- queries for this chunk: direct from x, no AG dep
                    qc_sb = pq.tile([P, NH, SC], BF16, tag="qc", bufs=4)
                    qr_sb = pq.tile([P, 2, SC], BF16, tag="qr", bufs=4)
                    q_tiles.append((qc_sb, qr_sb))
                    for m in range(NH):
                        ps = psA.tile([P, SC], F32, tag="mm")
                        for k in range(NKT):
                            nc.tensor.matmul(
                                ps[:],
                                wq_t[:, k, m * P : (m + 1) * P],
                                xt[:, k, :],
                                start=(k == 0),
                                stop=(k == NKT - 1),
                            )
                        nc.vector.tensor_scalar_add(
                            qc_sb[:, m, :], ps[:], bq_sb[:, m : m + 1]
                        )
                    for t in range(4):
                        rope_tokmajor(qc * 4 + t, NKT, xt, t, wqr_t,
                                      bqrT_sb, qr_sb, t)

                    # ---- keys/values for this chunk (needs AllGather qc)
                    for m in range(NH):
                        ps = psA.tile([P, SC], F32, tag="mm")
                        for k in range(NKV):
                            nc.tensor.matmul(
                                ps[:],
                                wku_t[:, k, m * P : (m + 1) * P],
                                kv_sb[:, k, qsl],
                                start=(k == 0),
                                stop=(k == NKV - 1),
                            )
                        nc.vector.tensor_scalar_add(
                            kc_sb[:, m, qsl], ps[:], bku_sb[:, m : m + 1]
                        )
                    for t in range(4):
                        tt = qc * 4 + t
                        ps = psA.tile([P, NH * HEAD_DIM], F32, tag="mm")
                        for k in range(NKV):
                            nc.tensor.matmul(
                                ps[:],
                                kv_sb[:, k, tt * P : (tt + 1) * P],
                                wvu_t[:, k, :],
                                start=(k == 0),
                                stop=(k == NKV - 1),
                            )
                        nc.vector.tensor_copy(v_sb[:, tt, :], ps[:])
                        rope_tokmajor(tt, NKV, kv_sb, tt, wkr_t,
                                      bkrT_sb, kr_sb, tt)

                # --------- phase 3: attention + out-projection ------------
                for qc in range(NSC):
                    qsl = slice(qc * SC, (qc + 1) * SC)
                    qc_sb, qr_sb = q_tiles[qc]
                    nkb = 4 * qc + 4
                    ctx_q = patx.tile([P, NH, SC], BF16, tag="ctx")
                    for h in range(NH):
                        hc = h // 2
                        hp = (h % 2) * ROPE_DIM
                        psum_ctx = psB.tile([P, SC], F32, tag="ctx")
                        acc = patt.tile([P, SC], BF16, tag="acc")
                        for kb in range(nkb):
                            ksl = slice(kb * P, (kb + 1) * P)
                            diag = kb >= 4 * qc
                            c = (kb - 4 * qc) * P if diag else 0
                            qs0 = qc * SC + c
                            ps = psA.tile([P, SC], F32, tag="sc")
                            nc.tensor.matmul(
                                ps[:, c:],
                                kc_sb[:, h, ksl],
                                qc_sb[:, h, c:],
                                start=True, stop=False,
                            )
                            nc.tensor.matmul(
                                ps[:, c:],
                                kr_sb[hp : hp + ROPE_DIM, hc, ksl],
                                qr_sb[hp : hp + ROPE_DIM, hc, c:],
                                start=False, stop=True,
                            )
                            probs = pat.tile([P, SC], BF16, tag="probs")
                            if diag:
                                nc.vector.tensor_tensor(
                                    ps[:, c : c + P],
                                    ps[:, c : c + P],
                                    tri_sb[:],
                                    mybir.AluOpType.add,
                                )
                            nc.scalar.activation(
                                probs[:, c:], ps[:, c:], AF.Exp, scale=SCALE
                            )
                            if kb == 0:
                                nc.vector.tensor_copy(acc[:], probs[:])
                            else:
                                nc.vector.tensor_tensor(
                                    acc[:, c:], acc[:, c:], probs[:, c:],
                                    mybir.AluOpType.add,
                                )
                            nc.tensor.matmul(
                                psum_ctx[:, c:],
                                v_sb[:, kb, h * P : (h + 1) * P],
                                probs[:, c:],
                                start=(kb == 0), stop=(kb == nkb - 1),
                            )
                        psd = psB.tile([P, SC], F32, tag="m")
                        nc.tensor.matmul(
                            psd[:], ones_sb[:], acc[:], start=True, stop=True
                        )
                        rcp = patt.tile([P, SC], F32, tag="rcp")
                        nc.vector.reciprocal(rcp[:], psd[:])
                        nc.vector.tensor_tensor(
                            ctx_q[:, h, :], psum_ctx[:], rcp[:],
                            mybir.AluOpType.mult,
                        )

                    # ---- out-projection for this q-chunk ----
                    for m in range(NKT):
                        wo_t = pow_.tile([P, NH, P], BF16, tag="wo")
                        nc.gpsimd.dma_start(
                            wo_t[:],
                            Wo[:, m * P : (m + 1) * P].rearrange(
                                "(t p) m -> p t m", p=P
                            ),
                        )
                        ps = psB.tile([P, SC], F32, tag="m")
                        for k in range(NH):
                            nc.tensor.matmul(
                                ps[:],
                                wo_t[:, k, :],
                                ctx_q[:, k, :],
                                start=(k == 0),
                                stop=(k == NH - 1),
                            )
                        og = pout.tile([P, SC], BF16, tag="og")
                        nc.scalar.copy(og[:], ps[:])
                        nc.sync.dma_start(
                            outT[m * P : (m + 1) * P, qsl], og[:]
                        )
    _split_waits(nc)
    return nc


def _col_bias(b, nm):
    """[nm*128] -> [128, nm] (column m = bias for feature chunk m)."""
    return np.ascontiguousarray(b.reshape(nm, P).T).astype(np.float32)


_NC = None


def kernel(**inputs):
    global _NC
    inp = {k: np.asarray(v) for k, v in inputs.items()}
    x = inp["x"].astype(np.float32)

    # token-major cos/sin: cosT[p, (t, h, i)] = cos(pos(t*128+p)*inv_freq[i])
    pos = np.arange(S, dtype=np.float64)
    inv = 1.0 / (10000.0 ** (np.arange(0, ROPE_DIM, 2, np.float64) / ROPE_DIM))
    ang = pos[:, None] * inv[None, :]              # [S, 32]
    cosS = np.cos(ang).reshape(NT, P, ROPE_DIM // 2)  # [t, p, i]
    sinS = np.sin(ang).reshape(NT, P, ROPE_DIM // 2)
    cosT = np.repeat(
        cosS.transpose(1, 0, 2)[:, :, None, :], NH, axis=2
    ).reshape(P, -1).astype(BF)
    sinT = np.repeat(
        sinS.transpose(1, 0, 2)[:, :, None, :], NH, axis=2
    ).reshape(P, -1).astype(BF)
    tri = np.where(
        np.arange(P)[None, :] >= np.arange(P)[:, None], 0.0, NEG
    ).astype(np.float32)
    eye = np.eye(P, dtype=np.float32).astype(BF)

    qdw = inp["query_down_w"].astype(np.float32)
    qdb = inp["query_down_b"].astype(np.float32)

    in_maps = []
    for c in range(8):
        b, r = c // 4, c % 4
        h0 = r * NH
        csl = slice(h0 * HEAD_DIM, (h0 + NH) * HEAD_DIM)
        rsl = slice(h0 * ROPE_DIM, (h0 + NH) * ROPE_DIM)
        # fold the query path: Q = x @ (qdw @ qu) + (qdb @ qu + qub)
        wq = qdw @ inp["query_up_w"][:, csl].astype(np.float32)
        bq_f = qdb @ inp["query_up_w"][:, csl].astype(np.float32) \
            + inp["query_up_b"][csl].astype(np.float32)
        wqr = qdw @ inp["query_rope_w"][:, rsl].astype(np.float32)
        bqr_f = qdb @ inp["query_rope_w"][:, rsl].astype(np.float32) \
            + inp["query_rope_b"][rsl].astype(np.float32)
        own_cols = np.concatenate(
            [np.arange(SC * j + P * r, SC * j + P * r + P) for j in range(NSC)]
        )
        in_maps.append(
            {
                "xT": np.ascontiguousarray(x[b].T).astype(BF),
                "xTo": np.ascontiguousarray(x[b].T[:, own_cols]).astype(BF),
                "Wdkv": inp["kv_down_w"].astype(BF),
                "bdkv": _col_bias(inp["kv_down_b"], NKV),
                "Wku": inp["key_up_w"][:, csl].astype(BF),
                "bku": _col_bias(inp["key_up_b"][csl], 4),
                "Wvu": inp["value_up_w"][:, csl].astype(BF),
                "Wkr": inp["key_rope_w"][:, rsl].astype(BF),
                "Wq": wq.astype(BF),
                "bq": _col_bias(bq_f, 4),
                "Wqr": wqr.astype(BF),
                "bkrT": np.broadcast_to(
                    inp["key_rope_b"][rsl].astype(np.float32),
                    (P, NH * ROPE_DIM),
                ).copy(),
                "bqrT": np.broadcast_to(
                    bqr_f, (P, NH * ROPE_DIM)
                ).copy(),
                "Wo": inp["out_w"][csl, :].astype(BF),
                "cosT": cosT,
                "sinT": sinT,
                "tri": tri,
                "eye": eye,
            }
        )

    if _NC is None:
        _NC = build()
    res = run_bass_kernel_spmd(_NC, in_maps, core_ids=list(range(8)))

    corr = (
        inp["value_up_b"].astype(np.float32) @ inp["out_w"].astype(np.float32)
        + inp["out_b"].astype(np.float32)
    )
    out = np.empty((B, S, HIDDEN), np.float32)
    for b in range(B):
        acc = res.results[b * 4]["outT"].astype(np.float32)
        for g in range(1, 4):
            acc += res.results[b * 4 + g]["outT"].astype(np.float32)
        out[b] = acc.T + corr[None, :]
    return out
